# revision 6
# baseline (speedup 1.0000x reference)
"""Single-head attention (shared QKV weight) on 8 Trainium2 NeuronCores.

Problem: B=4, S=2048, D=E=1024
  Q = xq@Wq.T + bq ; K = xk@Wq.T + bq ; V = xv@Wq.T + bq
  out = softmax(mask(Q@K.T/sqrt(E))) @ V

Sharding: data-parallel over batch x query-halves -> 8 cores. Core c
handles batch b=c//2 and a causally-balanced set of 8 query tiles (128
rows each) so every core executes the same instruction stream with the
same FLOP count. Each core computes the full K/V projection of its
batch (replicated within the batch pair), its own Q projection, and
attention for its query tiles.

Math shortcuts (exact):
- K-bias adds a per-query constant to every score row -> cancels in
  softmax -> skipped.
- Q-bias is fused into the Q-projection PSUM eviction (per-partition
  bias in the e-major layout).
- V-bias: rows of softmax sum to 1, so out = P@Vraw/rowsum + bq; added
  once to the output tile.
- Scores are bounded (|s|/32 <~ 12 for unit-normal inputs), so softmax
  skips the max-subtraction; exp never overflows fp32 and the
  normalizer is applied to the PV output via a per-partition scale.

All matmuls run in float32r (4x the fp32 throughput, ~1.5e-4 rel err).
"""

import re

import numpy as np

import concourse.bass as bass
import concourse.mybir as mybir
import concourse.tile as tile
from concourse.masks import make_identity
from concourse.vector_clock import ScopedClock

F32 = mybir.dt.float32
F32R = mybir.dt.float32r
AF = mybir.ActivationFunctionType

B, S, D, E = 4, 2048, 1024, 1024
NCORES = 8
SCALE = 1.0 / 32.0  # E ** -0.5
NEG = -1.0e30

# Causally balanced q-tile assignment: global tile t (128 rows) needs
# keys up to kend = 512*ceil((t+1)/4). Halves get the same multiset of
# kend classes so the SPMD program is identical across cores.
TILES_H0 = [0, 1, 4, 5, 8, 9, 12, 13]
TILES_H1 = [2, 3, 6, 7, 10, 11, 14, 15]

# ---------------------------------------------------------------------------
# Workarounds for this container's walrus build, which rejects any
# instruction carrying more than one semaphore wait.
# ---------------------------------------------------------------------------

_split_counter = [0]


def _legalize_waits(nc):
    """Move all-but-one sem wait from each instruction onto single-wait
    NoOps inserted immediately before it on the same engine. Engines
    dispatch in order, so the nops' waits are satisfied before the
    instruction issues."""
    for f in nc.m.functions:
        for bb in f.blocks:
            insts = list(bb.instructions)
            out = []
            changed = False
            for inst in insts:
                si = inst.sync_info
                if si is not None and si.on_wait is not None and len(si.on_wait) > 1:
                    waits = list(si.on_wait)
                    for w in waits[:-1]:
                        _split_counter[0] += 1
                        nop = mybir.InstNoOp(
                            name=f"I-waitsplit-{_split_counter[0]}",
                            opcode="NoOp",
                            engine=inst.engine,
                            sync_info=mybir.SyncInfo(on_wait=[w], on_update=[]),
                        )
                        nc.register_instruction(nop)
                        out.append(nop)
                    si.on_wait = [waits[-1]]
                    changed = True
                out.append(inst)
            if changed:
                bb.instructions = out


class _TileContext(tile.TileContext):
    def __init__(self, nc, **kw):
        kw.setdefault("pool_alloc_mode", "queue")
        super().__init__(nc, **kw)

    def _drain_and_barrier(self, tick_clock, wait_clock):
        gc = tick_clock.global_clock
        m = re.search(r"\[([0-9, ]*)\]", repr(gc))
        ticks = (
            [int(x) for x in m.group(1).split(",")]
            if m and m.group(1).strip()
            else []
        )
        for p, t in [(i, t) for i, t in enumerate(ticks) if t > 0]:
            nop = self.nc.sync.nop(nofuse=True, hint="drain_split")
            sc = ScopedClock({})
            sc.require_at_least(None, p, t)
            wait_clock.add_sem_waits(nop.ins, sc)
        self.nc.sync.drain()
        self.nc.all_engine_barrier()
        assert self.sems is not None
        popped = self.nc._tile_sem_poison_stack.pop()
        assert popped is self._sem_poison
        self.nc.clear_and_free_semaphores(list(self.sems.allocated().values()))
        self.nc.all_engine_barrier()

    def __exit__(self, *args):
        r = super().__exit__(*args)
        _legalize_waits(self.nc)
        return r


# ---------------------------------------------------------------------------
# Device program (identical on all 8 cores).
# ---------------------------------------------------------------------------


def build_program(chunk_counts, mask_chunks):
    """chunk_counts: per q-tile number of 512-wide key chunks to process.
    mask_chunks: set of (q_tile_idx, chunk_idx) that get an additive mask
    tile (ordered mask DRAM array follows this order)."""
    nmask = len(mask_chunks)
    mask_order = {qc: i for i, qc in enumerate(sorted(mask_chunks))}

    nc = bass.Bass("TRN2", target_bir_lowering=False, debug=False)
    wqT = nc.declare_dram_parameter("wqT", [D, E], F32R, isOutput=False)
    xqT = nc.declare_dram_parameter("xqT", [D, 1024], F32R, isOutput=False)
    xkT = nc.declare_dram_parameter("xkT", [D, S], F32R, isOutput=False)
    xvT = nc.declare_dram_parameter("xvT", [D, S], F32R, isOutput=False)
    bq8 = nc.declare_dram_parameter("bq8", [128, 8], F32, isOutput=False)
    bqb = nc.declare_dram_parameter("bqb", [128, E], F32, isOutput=False)
    if nmask:
        maskd = nc.declare_dram_parameter(
            "maskd", [nmask, 128, 512], F32, isOutput=False
        )
    out = nc.declare_dram_parameter("out", [1024, E], F32, isOutput=True)

    with _TileContext(nc) as tc:
        with (
            tc.tile_pool(name="const", bufs=1) as cpool,
            tc.tile_pool(name="big", bufs=1) as bpool,
        ):
            wq_ctx = tc.tile_pool(name="wqpool", bufs=1)
            wqpool = wq_ctx.__enter__()
            wq_sb = wqpool.tile([128, 8, E], F32R, tag="wq")
            nc.sync.dma_start(wq_sb[:], wqT.ap().rearrange("(t p) e -> p t e", p=128))
            bq8_sb = cpool.tile([128, 8], F32, tag="bq8")
            nc.sync.dma_start(bq8_sb[:], bq8[:])
            bqb_sb = cpool.tile([128, E], F32, tag="bqb")
            nc.sync.dma_start(bqb_sb[:], bqb[:])
            ident = cpool.tile([128, 128], F32, tag="ident")
            make_identity(nc, ident[:])

            q_sb = bpool.tile([128, 8, 1024], F32R, tag="q")
            k_sb = bpool.tile([128, 8, S], F32R, tag="k")
            v_sb = bpool.tile([128, 16, E], F32R, tag="v")

            # ---- projections ----
            with (
                tc.tile_pool(name="pstage", bufs=2) as stpool,
                tc.tile_pool(name="vstage", bufs=1) as vstpool,
                tc.tile_pool(name="projps", bufs=8, space="PSUM") as ppsum,
            ):
                # Q^T and K^T (e-major): out[e, s] += WqT[d, e].T @ xT[d, s]
                for xT, dst, nch, with_bias in (
                    (xqT, q_sb, 2, True),
                    (xkT, k_sb, 4, False),
                ):
                    for ch in range(nch):
                        pss = [
                            ppsum.tile([128, 512], F32, tag="pp", name=f"pp{i}")
                            for i in range(8)
                        ]
                        for dt in range(8):
                            xst = stpool.tile([128, 512], F32R, tag="xst")
                            nc.sync.dma_start(
                                xst[:],
                                xT[
                                    dt * 128 : (dt + 1) * 128,
                                    ch * 512 : (ch + 1) * 512,
                                ],
                            )
                            for et in range(8):
                                nc.tensor.matmul(
                                    pss[et][:],
                                    wq_sb[:, dt, et * 128 : (et + 1) * 128],
                                    xst[:],
                                    start=(dt == 0),
                                    stop=(dt == 7),
                                )
                        for et in range(8):
                            if with_bias:
                                nc.scalar.activation(
                                    dst[:, et, ch * 512 : (ch + 1) * 512],
                                    pss[et][:],
                                    AF.Identity,
                                    bias=bq8_sb[:, et : et + 1],
                                )
                            else:
                                nc.scalar.activation(
                                    dst[:, et, ch * 512 : (ch + 1) * 512],
                                    pss[et][:],
                                    AF.Copy,
                                )

                # V (s-major): out[s, e] += xvT[d, s].T @ WqT[d, e]
                for st in range(16):
                    xst = vstpool.tile([128, 8, 128], F32R, tag="xvst")
                    nc.sync.dma_start(
                        xst[:],
                        xvT.ap().rearrange("(t p) s -> p t s", p=128)[
                            :, :, st * 128 : (st + 1) * 128
                        ],
                    )
                    for ec in range(2):
                        ps = ppsum.tile([128, 512], F32, tag="pp")
                        for dt in range(8):
                            nc.tensor.matmul(
                                ps[:],
                                xst[:, dt, :],
                                wq_sb[:, dt, ec * 512 : (ec + 1) * 512],
                                start=(dt == 0),
                                stop=(dt == 7),
                            )
                        nc.vector.tensor_copy(
                            v_sb[:, st, ec * 512 : (ec + 1) * 512], ps[:]
                        )

            # ---- attention ----
            wq_ctx.__exit__(None, None, None)
            with (
                tc.tile_pool(name="work", bufs=3) as wpool,
                tc.tile_pool(name="small", bufs=4) as spool,
                tc.tile_pool(name="mstage", bufs=2) as mpool,
                tc.tile_pool(name="opool", bufs=2) as opool,
                tc.tile_pool(name="sps", bufs=2, space="PSUM") as spsum,
                tc.tile_pool(name="trps", bufs=2, space="PSUM") as trpsum,
                tc.tile_pool(name="ops", bufs=2, space="PSUM") as opsum,
            ):
                for qt in range(8):
                    ncha = chunk_counts[qt]
                    o_ps = opsum.tile([128, 1024], F32, tag="o")
                    rs = spool.tile([128, 1], F32, tag="rs")
                    for kc in range(ncha):
                        s_ps = spsum.tile([128, 512], F32, tag="s")
                        for et in range(8):
                            nc.tensor.matmul(
                                s_ps[:],
                                q_sb[:, et, qt * 128 : (qt + 1) * 128],
                                k_sb[:, et, kc * 512 : (kc + 1) * 512],
                                start=(et == 0),
                                stop=(et == 7),
                            )
                        if (qt, kc) in mask_order:
                            msk = mpool.tile([128, 512], F32, tag="msk")
                            nc.sync.dma_start(msk[:], maskd[mask_order[(qt, kc)]])
                            nc.vector.tensor_add(s_ps[:], s_ps[:], msk[:])
                        p_sb = wpool.tile([128, 512], F32, tag="p")
                        part = spool.tile([128, 1], F32, tag="part")
                        nc.scalar.activation(
                            p_sb[:],
                            s_ps[:],
                            AF.Exp,
                            scale=SCALE,
                            accum_out=part[:],
                        )
                        if kc == 0:
                            nc.vector.tensor_copy(rs[:], part[:])
                        else:
                            nc.vector.tensor_add(rs[:], rs[:], part[:])
                        pT = wpool.tile([128, 512], F32R, tag="pt")
                        for j in range(4):
                            tr_ps = trpsum.tile([128, 128], F32, tag="tr")
                            nc.tensor.transpose(
                                tr_ps[:], p_sb[:, j * 128 : (j + 1) * 128], ident[:]
                            )
                            nc.vector.tensor_copy(
                                pT[:, j * 128 : (j + 1) * 128], tr_ps[:]
                            )
                        for j in range(4):
                            kidx = kc * 4 + j
                            for ec in range(2):
                                nc.tensor.matmul(
                                    o_ps[:, ec * 512 : (ec + 1) * 512],
                                    pT[:, j * 128 : (j + 1) * 128],
                                    v_sb[:, kidx, ec * 512 : (ec + 1) * 512],
                                    start=(kidx == 0),
                                    stop=(kidx == ncha * 4 - 1),
                                )
                    rcp = spool.tile([128, 1], F32, tag="rcp")
                    nc.vector.reciprocal(rcp[:], rs[:])
                    o_sb = opool.tile([128, E], F32, tag="osb")
                    nc.scalar.activation(o_sb[:], o_ps[:], AF.Copy, scale=rcp[:])
                    nc.vector.tensor_add(o_sb[:], o_sb[:], bqb_sb[:])
                    nc.sync.dma_start(out[qt * 128 : (qt + 1) * 128, :], o_sb[:])

    return nc


# ---------------------------------------------------------------------------
# Host wrapper.
# ---------------------------------------------------------------------------

_prog_cache = {}


def _get_program(variant, chunk_counts, mask_chunks):
    key = (variant, tuple(chunk_counts), tuple(sorted(mask_chunks)))
    if key not in _prog_cache:
        _prog_cache[key] = build_program(chunk_counts, mask_chunks)
    return _prog_cache[key]


def _analyze_mask(att_mask):
    """Return (chunk_counts per local tile slot, mask_chunks, tiles maps)."""
    causal = np.array_equal(
        att_mask, np.triu(np.ones((S, S), dtype=att_mask.dtype), 1)
    )
    if causal:
        # local slot i covers global tile TILES_H*[i]; kend class per slot
        chunk_counts = [1, 1, 2, 2, 3, 3, 4, 4]
        mask_chunks = {(qt, chunk_counts[qt] - 1) for qt in range(8)}
        return "causal", chunk_counts, mask_chunks
    if not att_mask.any():
        return "nomask", [4] * 8, set()
    return "generic", [4] * 8, {(qt, kc) for qt in range(8) for kc in range(4)}


def kernel(xq, xk, xv, Wq, bq, att_mask):
    from concourse.bass_utils import run_bass_kernel_spmd

    variant, chunk_counts, mask_chunks = _analyze_mask(np.asarray(att_mask))
    nc = _get_program(variant, chunk_counts, mask_chunks)

    xq = np.asarray(xq, dtype=np.float32)
    xk = np.asarray(xk, dtype=np.float32)
    xv = np.asarray(xv, dtype=np.float32)
    Wq = np.asarray(Wq, dtype=np.float32)
    bq = np.asarray(bq, dtype=np.float32)

    wqT = np.ascontiguousarray(Wq.T)  # [d, e]
    bq8 = np.ascontiguousarray(bq.reshape(8, 128).T)  # [128, 8]
    bqb = np.ascontiguousarray(np.broadcast_to(bq, (128, E)))

    mask_list = sorted(mask_chunks)
    tiles_by_half = (TILES_H0, TILES_H1)

    in_maps = []
    for c in range(NCORES):
        b, h = divmod(c, 2)
        tiles = tiles_by_half[h]
        rows = np.concatenate(
            [np.arange(t * 128, (t + 1) * 128) for t in tiles]
        )
        m = {
            "wqT": wqT,
            "xqT": np.ascontiguousarray(xq[b].T[:, rows]),
            "xkT": np.ascontiguousarray(xk[b].T),
            "xvT": np.ascontiguousarray(xv[b].T),
            "bq8": bq8,
            "bqb": bqb,
        }
        if mask_list:
            md = np.empty((len(mask_list), 128, 512), dtype=np.float32)
            for i, (qt, kc) in enumerate(mask_list):
                t = tiles[qt]
                md[i] = att_mask[
                    t * 128 : (t + 1) * 128, kc * 512 : (kc + 1) * 512
                ].astype(np.float32) * NEG
            m["maskd"] = md
        in_maps.append(m)

    res = run_bass_kernel_spmd(nc, in_maps, list(range(NCORES)))

    out = np.empty((B, S, E), dtype=np.float32)
    for c in range(NCORES):
        b, h = divmod(c, 2)
        tiles = tiles_by_half[h]
        oc = res.results[c]["out"]
        for i, t in enumerate(tiles):
            out[b, t * 128 : (t + 1) * 128, :] = oc[i * 128 : (i + 1) * 128, :]
    return out
